# revision 20
# baseline (speedup 1.0000x reference)
"""BERT self-attention on 8 Trainium2 NeuronCores.

Sharding: data-parallel over batch (B=8 -> 1 batch element per core).
Every core runs the same single-core Bass kernel on its own batch slice;
weights/mask are replicated. The final output is a host-side stack.

Per-core algorithm (S=1024, HID=1024, NH=16, HD=64), matmuls bf16 with
fp32 PSUM accumulation:

  Q^T/K^T = W^T @ X^T (+bias)  [HID, S]; V = X @ Wv (+bv)
  per head pair (2c, 2c+1) in hid chunk c (head 2c in partitions 0:64,
  head 2c+1 in 64:128 of the qT/kT chunk):
    S^T via 64x64-stationary PE-tiled matmuls: per (head, q-half) two
    col-tiled tiles at tile_position (r,0)/(r,64) share the moving Q
    stream; the two heads' row groups run concurrently -> a whole kt
    block (8 matmuls) issues in ~600ns.
    P^T = exp(S^T/8 + mask[k]) on ScalarE -- the bottleneck engine:
    128 blocks x ~1.1us = ~142us; the schedule keeps this stream dense.
    (A Pool-engine polynomial offload was tried and reverted: Pool
    tensor ops run at 2.4-17us per [128,1024] block, far slower than
    ScalarE's exp.)
    ctx = P^T.T @ [V_h | 1]  (ones column = softmax denominator)
    out[:, pair] = ctx * (1/Z), one batched 512KB DMA per head pair.

Scheduling: everything is emitted in per-kt "slots" sized ~1us so the
in-order PE queue never buries the score->exp stream: QK(c+1) in
quarter-groups (4 matmuls), V in (seq-chunk x 4-head) quarter-width
groups placed strictly before their consuming ctx chunk, ctx(c-1) in
the second-half slots of chunk c. wq/wk are DMA'd as column chunks so
Q(0)/K(0) start right after xT lands; warmup matmuls keep the PE clock
ramped through the DMA fill.
"""

import functools

import numpy as np
import ml_dtypes

B, S, HID = 8, 1024, 1024
NH, HD = 16, 64
P = 128
NCH = HID // P  # hid chunks (8)
NKT = S // P  # key tiles (8)
NQT = S // P  # query tiles (8)
VROW = NH * (HD + 1)  # 1040: per-seq-chunk V row: 16 x (64 V cols + ones col)
N_CORES = 8

SCALE = 1.0 / float(np.sqrt(HD))


@functools.lru_cache(maxsize=None)
def _build(has_bv: bool):
    import concourse.bass as bass
    import concourse.tile as tile
    from concourse import bacc, mybir
    from contextlib import ExitStack

    fp32 = mybir.dt.float32
    bf16 = mybir.dt.bfloat16
    EXP = mybir.ActivationFunctionType.Exp

    nc = bacc.Bacc("TRN2", target_bir_lowering=False)

    xT = nc.dram_tensor("xT", [HID, S], bf16, kind="ExternalInput")
    wq = nc.dram_tensor("wq", [NCH, P, NCH, P], bf16, kind="ExternalInput")
    wk = nc.dram_tensor("wk", [NCH, P, NCH, P], bf16, kind="ExternalInput")
    wv = nc.dram_tensor("wv", [HID, HID], bf16, kind="ExternalInput")
    bq = nc.dram_tensor("bq", [P, NCH], fp32, kind="ExternalInput")
    bk = nc.dram_tensor("bk", [P, NCH], fp32, kind="ExternalInput")
    bv = nc.dram_tensor("bv", [HID], fp32, kind="ExternalInput") if has_bv else None
    mask = nc.dram_tensor("mask", [P, NKT], fp32, kind="ExternalInput")
    out = nc.dram_tensor("out", [S, HID], fp32, kind="ExternalOutput")

    with tile.TileContext(nc) as tc, ExitStack() as ctx:
        persist = ctx.enter_context(tc.tile_pool(name="persist", bufs=1))
        misc = ctx.enter_context(tc.tile_pool(name="misc", bufs=8))
        qT_pool = ctx.enter_context(tc.tile_pool(name="qT", bufs=2))
        kT_pool = ctx.enter_context(tc.tile_pool(name="kT", bufs=2))
        pT_pool = ctx.enter_context(tc.tile_pool(name="pT", bufs=4))
        out_pool = ctx.enter_context(tc.tile_pool(name="out", bufs=2))
        # PSUM budget (8 banks): scores 3x[128,1024] (6 banks) give the
        # score matmuls ~1.5 exps of lookahead so the exp stream paces at
        # pure ACT rate; QK/V/ctx groups share one 2-bank pool (their
        # quarter jobs are emitted adjacently, so at most two groups are
        # ever live)
        qkv_ps = ctx.enter_context(tc.tile_pool(name="qkv_ps", bufs=2, space="PSUM"))
        sc_ps = ctx.enter_context(tc.tile_pool(name="sc_ps", bufs=3, space="PSUM"))

        # ---- persistent SBUF tensors ----
        xT_c = [persist.tile([P, S], bf16, name=f"xT{c}") for c in range(NCH)]
        wq_c = [persist.tile([P, NCH, P], bf16, name=f"wq{c}") for c in range(NCH)]
        wk_c = [persist.tile([P, NCH, P], bf16, name=f"wk{c}") for c in range(NCH)]
        wv_c = [persist.tile([P, HID], bf16, name=f"wv{c}") for c in range(NCH)]
        v_sb = persist.tile([P, NKT, VROW], bf16)
        bq_sb = persist.tile([P, NCH], fp32)
        bk_sb = persist.tile([P, NCH], fp32)
        mask_sb = persist.tile([P, NKT], fp32)
        bv_sb = persist.tile([P, HID], fp32, name="bv_sb") if has_bv else None

        # ---- input DMAs, latency-ordered ----
        nc.sync.dma_start(out=bq_sb, in_=bq[:, :])
        nc.sync.dma_start(out=bk_sb, in_=bk[:, :])
        nc.sync.dma_start(out=mask_sb, in_=mask[:, :])
        if has_bv:
            bv_bcast = bass.AP(tensor=bv.tensor if hasattr(bv, "tensor") else bv,
                               offset=0, ap=[[0, P], [1, HID]])
            nc.sync.dma_start(out=bv_sb, in_=bv_bcast)
        for c in range(NCH):
            nc.sync.dma_start(out=xT_c[c], in_=xT[c * P:(c + 1) * P, :])
        nc.sync.dma_start(out=wq_c[0], in_=wq[0])
        nc.sync.dma_start(out=wk_c[0], in_=wk[0])
        # remaining weight DMAs are issued inside the early chunk slots so
        # xT (the critical path to the first exp) has the full HBM
        # bandwidth to itself during the fill
        # staged order mirrors consumption: wq1/wk1 feed the QK(1)
        # filler jobs in chunk 0's first slots, wv feeds the V jobs in
        # chunk 0's second-half slots, later columns feed later chunks
        late_dmas = [(wq_c[1], wq[1]), (wk_c[1], wk[1])]
        for c in range(NCH):
            late_dmas.append((wv_c[c], wv[c * P:(c + 1) * P, :]))
        for c in range(2, NCH):
            late_dmas.append((wq_c[c], wq[c]))
            late_dmas.append((wk_c[c], wk[c]))

        # ones columns for the softmax denominator live at col 64 of each
        # 65-wide head block; V copies below only overwrite cols 0..63
        nc.gpsimd.memset(v_sb, 1.0)

        # warmup matmuls on scratch data: keep the PE busy (clock ramp)
        # until xT + the first weight columns land
        wscr = persist.tile([P, 512], bf16, name="warm_scratch")
        nc.vector.memset(wscr, 0.5)
        for _ in range(16):
            wps = sc_ps.tile([P, S], fp32, name="score_psum")
            nc.tensor.matmul(
                wps[:, 0:512],
                lhsT=wscr[:, 0:P],
                rhs=wscr,
                start=True,
                stop=True,
            )

        qT_tiles = {}
        kT_tiles = {}
        qk_open = {}

        def qk_quarter(c, w_c, b_sb, dst_tiles, half, quarter):
            # one quarter (4 contraction chunks) of a Q/K projection half;
            # quarter 1 finishes the group and drains (+bias) to SBUF.
            # Quarters of one half are adjacent in the job list so the
            # qkv_ps rotation never sees two open groups.
            if c not in dst_tiles:
                pool = qT_pool if dst_tiles is qT_tiles else kT_pool
                dst_tiles[c] = pool.tile([P, S], bf16, name="qkT")
            key = (id(dst_tiles), c, half)
            if quarter == 0:
                qk_open[key] = qkv_ps.tile([P, 512], fp32, name="qkv_psum")
            ps = qk_open[key]
            for kc in range(4 * quarter, 4 * quarter + 4):
                nc.tensor.matmul(
                    ps,
                    lhsT=w_c[c][:, kc, :],
                    rhs=xT_c[kc][:, half * 512:(half + 1) * 512],
                    start=(kc == 0),
                    stop=(kc == NCH - 1),
                )
            if quarter == 1:
                nc.vector.tensor_scalar_add(
                    out=dst_tiles[c][:, half * 512:(half + 1) * 512],
                    in0=ps,
                    scalar1=b_sb[:, c:c + 1],
                )
                del qk_open[key]

        def v_quarter(st, w):
            # v_sb[:, st, heads 4w..4w+3] = (X @ Wv)[st, 256-col quarter w]
            ps = qkv_ps.tile([P, 512], fp32, name="qkv_psum")[:, 0:256]
            for kc in range(NCH):
                nc.tensor.matmul(
                    ps,
                    lhsT=xT_c[kc][:, st * P:(st + 1) * P],
                    rhs=wv_c[kc][:, w * 256:(w + 1) * 256],
                    start=(kc == 0),
                    stop=(kc == NCH - 1),
                )
            dst = (
                v_sb[:, st, :]
                .rearrange("p (h x) -> p h x", x=HD + 1)[:, w * 4:(w + 1) * 4, 0:HD]
            )
            src = ps.rearrange("p (h x) -> p h x", x=HD)
            if has_bv:
                bvs = (
                    bv_sb[:, w * 256:(w + 1) * 256]
                    .rearrange("p (h x) -> p h x", x=HD)
                )
                nc.vector.tensor_add(out=dst, in0=src, in1=bvs)
            else:
                nc.vector.tensor_copy(out=dst, in_=src)

        def score_block(c, kt):
            # scores for both heads of chunk c at key tile kt; returns the
            # two PSUM tiles. Per (head, q-half): two col-tiled 64x64
            # stationaries (keys 0:64 -> psum partitions 0:64 at tile col
            # 0; keys 64:128 -> partitions 64:128 at col 64) sharing the
            # moving Q stream; the two heads' row groups run concurrently.
            qT_t, kT_t = qT_tiles[c], kT_tiles[c]
            tiles = []
            for sub in range(2):
                po = 64 * sub
                ps = sc_ps.tile([P, S], fp32, name="score_psum")
                tiles.append(ps)
                for half in range(2):
                    for kg in range(2):
                        nc.tensor.matmul(
                            ps[kg * 64:(kg + 1) * 64, half * 512:(half + 1) * 512],
                            lhsT=kT_t[po:po + 64, kt * P + kg * 64:kt * P + (kg + 1) * 64],
                            rhs=qT_t[po:po + 64, half * 512:(half + 1) * 512],
                            start=True,
                            stop=True,
                        )
            return tiles

        def softmax_exp(ps, pT_h, kt):
            nc.scalar.activation(
                out=pT_h[:, kt, :],
                in_=ps,
                func=EXP,
                bias=mask_sb[:, kt:kt + 1],
                scale=SCALE,
            )

        def ctx_quarter(h, pT_h, pair_out, col, qt_base):
            # two qt context groups + normalization for head h; writes the
            # head-pair out tile at column block `col`
            for qt in (qt_base, qt_base + 1):
                cps = qkv_ps.tile([P, 512], fp32, name="qkv_psum")[:, 0:HD + 1]
                for kc in range(NKT):
                    nc.tensor.matmul(
                        cps,
                        lhsT=pT_h[:, kc, qt * P:(qt + 1) * P],
                        rhs=v_sb[:, kc, h * (HD + 1):(h + 1) * (HD + 1)],
                        start=(kc == 0),
                        stop=(kc == NKT - 1),
                    )
                recip = misc.tile([P, 1], fp32, name="recip")
                nc.vector.reciprocal(recip, cps[:, HD:HD + 1])
                nc.vector.tensor_scalar_mul(
                    out=pair_out[:, qt, col * HD:(col + 1) * HD],
                    in0=cps[:, 0:HD],
                    scalar1=recip,
                )

        def ctx_chunk_jobs(cc):
            # 8 ctx quarter-jobs for head pair (2cc, 2cc+1) + final DMA
            pA, pB = pT_live.pop(cc)
            pair_out = out_pool.tile([P, NQT, 2 * HD], fp32, name="pair_out")
            jobs = []
            for qt_base in range(0, NQT, 2):
                jobs.append(("ctx", (2 * cc, pA, pair_out, 0, qt_base)))
                jobs.append(("ctx", (2 * cc + 1, pB, pair_out, 1, qt_base)))
            jobs.append(("dma", (cc, pair_out)))
            return jobs

        def pair_dma(cc, pair_out):
            # one 512KB DMA for the head pair's output columns
            dst = (
                out[:, 2 * cc * HD:(2 * cc + 2) * HD]
                .rearrange("(qt p) c -> p qt c", p=P)
            )
            nc.sync.dma_start(out=dst, in_=pair_out)

        def run_job(job):
            kind, args = job
            if kind == "qk":
                qk_quarter(*args)
            elif kind == "v":
                v_quarter(*args)
            elif kind == "ctx":
                ctx_quarter(*args)
            else:
                pair_dma(*args)

        # ---- pipeline ----
        # V width-quarter w feeds heads 4w..4w+3, first consumed by
        # ctx(2w) during chunk 2w+1 -> schedule strictly before that.
        v_sched = {
            0: [(st, 0) for st in range(NKT)],
            1: [(st, 1) for st in range(4)],
            2: [(st, 1) for st in range(4, NKT)],
            3: [(st, 2) for st in range(4)],
            4: [(st, 2) for st in range(4, NKT)],
            5: [(st, 3) for st in range(4)],
            6: [(st, 3) for st in range(4, NKT)],
        }
        pT_live = {}
        dma_stage = list(late_dmas)

        # Q(0) + K(0) half 0 ahead of the stream (kt 0-3 only need K h0;
        # K h1 quarters are the first fillers of chunk 0)
        for half in range(2):
            for quarter in range(2):
                qk_quarter(0, wq_c, bq_sb, qT_tiles, half, quarter)
        for quarter in range(2):
            qk_quarter(0, wk_c, bk_sb, kT_tiles, 0, quarter)

        for c in range(NCH):
            pT_pair = (
                pT_pool.tile([P, NKT, S], bf16, name="pT"),
                pT_pool.tile([P, NKT, S], bf16, name="pT"),
            )
            pT_live[c] = pT_pair

            front = []  # spread over all 8 kt slots
            if c == 0:
                for quarter in range(2):
                    front.append(("qk", (0, wk_c, bk_sb, kT_tiles, 1, quarter)))
            if c + 1 < NCH:
                for w_c, b_sb, dst in ((wq_c, bq_sb, qT_tiles), (wk_c, bk_sb, kT_tiles)):
                    for half in range(2):
                        for quarter in range(2):
                            front.append(("qk", (c + 1, w_c, b_sb, dst, half, quarter)))
            v_c = [("v", vj) for vj in v_sched.get(c, [])]
            if c == 0:
                back = v_c  # wv DMAs stream in during the first half
            else:
                front.extend(v_c)
                back = ctx_chunk_jobs(c - 1)  # second-half slots

            per_kt = [[] for _ in range(NKT)]
            n = len(front)
            for kt in range(NKT):
                per_kt[kt] = front[kt * n // NKT:(kt + 1) * n // NKT]
            nb = len(back)
            for i, kt in enumerate(range(4, NKT)):
                per_kt[kt] += back[i * nb // 4:(i + 1) * nb // 4]

            for kt in range(NKT):
                # staged input DMAs must be emitted before this kt's
                # consumers (the framework orders same-region accesses by
                # emission order)
                for _ in range(2):
                    if dma_stage:
                        dst, src_ap = dma_stage.pop(0)
                        nc.sync.dma_start(out=dst, in_=src_ap)
                ps_A, ps_B = score_block(c, kt)
                softmax_exp(ps_A, pT_pair[0], kt)
                softmax_exp(ps_B, pT_pair[1], kt)
                for job in per_kt[kt]:
                    run_job(job)
            qT_tiles.pop(c)
            kT_tiles.pop(c)

        # tail: last head pair
        for job in ctx_chunk_jobs(7):
            run_job(job)

    nc.finalize()
    return nc


def _prep_inputs(inputs):
    bf16 = ml_dtypes.bfloat16
    hs = np.asarray(inputs["hidden_states"], dtype=np.float32)
    am = np.asarray(inputs["attention_mask"], dtype=np.float32)
    Wq = np.asarray(inputs["Wq"], dtype=np.float32)
    Wk = np.asarray(inputs["Wk"], dtype=np.float32)
    Wv = np.asarray(inputs["Wv"], dtype=np.float32)
    bq = np.asarray(inputs["bq"], dtype=np.float32)
    bk = np.asarray(inputs["bk"], dtype=np.float32)
    bv = np.asarray(inputs["bv"], dtype=np.float32)

    has_bv = bool(np.any(bv))

    # [hid_in, hid_out] -> [c_out, p(hid_in%128), kc(hid_in//128), col]
    def col_shuffle(w):
        return np.ascontiguousarray(
            w.astype(bf16).reshape(NCH, P, NCH, P).transpose(2, 1, 0, 3)
        )

    wq_b = col_shuffle(Wq)
    wk_b = col_shuffle(Wk)
    wv_b = np.ascontiguousarray(Wv.astype(bf16))
    bq_c = np.ascontiguousarray(bq.reshape(NCH, P).T)
    bk_c = np.ascontiguousarray(bk.reshape(NCH, P).T)

    hs_b = hs.astype(bf16)
    in_maps = []
    for b in range(B):
        m = {
            "xT": np.ascontiguousarray(hs_b[b].T),
            "wq": wq_b,
            "wk": wk_b,
            "wv": wv_b,
            "bq": bq_c,
            "bk": bk_c,
            "mask": np.ascontiguousarray(am[b, 0, 0].reshape(NKT, P).T),
        }
        if has_bv:
            m["bv"] = bv
        in_maps.append(m)
    return in_maps, has_bv


def _run(inputs, trace=False, trace_cores=None):
    from concourse.bass_utils import run_bass_kernel_spmd

    in_maps, has_bv = _prep_inputs(inputs)
    nc = _build(has_bv)
    res = run_bass_kernel_spmd(
        nc, in_maps, core_ids=list(range(N_CORES)), trace=trace,
        trace_cores=trace_cores,
    )
    out = np.stack([np.asarray(r["out"], dtype=np.float32) for r in res.results])
    return out, res


def kernel(**inputs) -> np.ndarray:
    out, _ = _run(inputs, trace=False)
    return out


# revision 21
# speedup vs baseline: 1.3438x; 1.3438x over previous
"""BERT self-attention on 8 Trainium2 NeuronCores.

Sharding: data-parallel over batch (B=8 -> 1 batch element per core).
Every core runs the same single-core Bass kernel on its own batch slice;
weights/mask are replicated. The final output is a host-side stack.

Per-core algorithm (S=1024, HID=1024, NH=16, HD=64), matmuls bf16 with
fp32 PSUM accumulation:

  Q^T/K^T = W^T @ X^T (+bias)  [HID, S]; V = X @ Wv (+bv)
  per head pair (2c, 2c+1) in hid chunk c (head 2c in partitions 0:64,
  head 2c+1 in 64:128 of the qT/kT chunk), heads processed sequentially:
    S^T = K_h @ Q_h^T with K^T zero-padded to 128 contraction rows
    (variant v holds head 2c+v's 64 rows at partition offset 64v, zeros
    elsewhere; the zero rows annihilate the other head's Q rows). One
    512-col matmul per (head, kt, q-half) keeps FWL enabled, and the
    2-buffer PSUM rotation within a head gives each score matmul two
    exps of lookahead, so the exp stream below never waits on PE latency.
    P^T = exp(S^T/8 + mask[k]) on ScalarE -- the bottleneck engine:
    128 blocks x ~1.1us = ~142us; everything else is scheduled around
    keeping this stream dense and starting it early.
    ctx = P^T.T @ [V_h | 1]  (ones column = softmax denominator)
    out[:, pair] = ctx * (1/Z), one batched 512KB DMA per head pair.

Pipeline: wq/wk are DMA'd as column chunks and xT gets the HBM to
itself during the fill (remaining weight DMAs are staged into the early
chunk slots), so Q(0)/K(0) and the first exp start ~21us in instead of
~59us. Warmup matmuls keep the PE clock ramped through the fill. Filler
work -- QK(c+1) quarter-groups, V quarter-width groups (placed strictly
before their consuming ctx chunk), ctx(c-1) -- is sliced into ~1us jobs
and emitted between the 16 per-chunk exp steps so the in-order PE queue
never buries the score->exp chain.
"""

import functools

import numpy as np
import ml_dtypes

B, S, HID = 8, 1024, 1024
NH, HD = 16, 64
P = 128
NCH = HID // P  # hid chunks (8)
NKT = S // P  # key tiles (8)
NQT = S // P  # query tiles (8)
VROW = NH * (HD + 1)  # 1040: per-seq-chunk V row: 16 x (64 V cols + ones col)
N_CORES = 8

SCALE = 1.0 / float(np.sqrt(HD))


@functools.lru_cache(maxsize=None)
def _build(has_bv: bool):
    import concourse.bass as bass
    import concourse.tile as tile
    from concourse import bacc, mybir
    from contextlib import ExitStack

    fp32 = mybir.dt.float32
    bf16 = mybir.dt.bfloat16
    EXP = mybir.ActivationFunctionType.Exp

    nc = bacc.Bacc("TRN2", target_bir_lowering=False)

    xT = nc.dram_tensor("xT", [HID, S], bf16, kind="ExternalInput")
    wq = nc.dram_tensor("wq", [NCH, P, NCH, P], bf16, kind="ExternalInput")
    wk = nc.dram_tensor("wk", [NCH, P, NCH, P], bf16, kind="ExternalInput")
    wv = nc.dram_tensor("wv", [HID, HID], bf16, kind="ExternalInput")
    bq = nc.dram_tensor("bq", [P, NCH], fp32, kind="ExternalInput")
    bk = nc.dram_tensor("bk", [P, NCH], fp32, kind="ExternalInput")
    bv = nc.dram_tensor("bv", [HID], fp32, kind="ExternalInput") if has_bv else None
    mask = nc.dram_tensor("mask", [P, NKT], fp32, kind="ExternalInput")
    out = nc.dram_tensor("out", [S, HID], fp32, kind="ExternalOutput")

    with tile.TileContext(nc) as tc, ExitStack() as ctx:
        persist = ctx.enter_context(tc.tile_pool(name="persist", bufs=1))
        misc = ctx.enter_context(tc.tile_pool(name="misc", bufs=8))
        qT_pool = ctx.enter_context(tc.tile_pool(name="qT", bufs=2))
        kT_pool = ctx.enter_context(tc.tile_pool(name="kT", bufs=2))
        pT_pool = ctx.enter_context(tc.tile_pool(name="pT", bufs=4))
        out_pool = ctx.enter_context(tc.tile_pool(name="out", bufs=2))
        qkv_ps = ctx.enter_context(tc.tile_pool(name="qkv_ps", bufs=2, space="PSUM"))
        sc_ps = ctx.enter_context(tc.tile_pool(name="sc_ps", bufs=2, space="PSUM"))
        cx_ps = ctx.enter_context(tc.tile_pool(name="cx_ps", bufs=2, space="PSUM"))

        # ---- persistent SBUF tensors ----
        xT_c = [persist.tile([P, S], bf16, name=f"xT{c}") for c in range(NCH)]
        wq_c = [persist.tile([P, NCH, P], bf16, name=f"wq{c}") for c in range(NCH)]
        wk_c = [persist.tile([P, NCH, P], bf16, name=f"wk{c}") for c in range(NCH)]
        wv_c = [persist.tile([P, HID], bf16, name=f"wv{c}") for c in range(NCH)]
        v_sb = persist.tile([P, NKT, VROW], bf16)
        bq_sb = persist.tile([P, NCH], fp32)
        bk_sb = persist.tile([P, NCH], fp32)
        mask_sb = persist.tile([P, NKT], fp32)
        bv_sb = persist.tile([P, HID], fp32, name="bv_sb") if has_bv else None

        # ---- input DMAs: xT + first weight columns get the HBM first ----
        nc.sync.dma_start(out=bq_sb, in_=bq[:, :])
        nc.sync.dma_start(out=bk_sb, in_=bk[:, :])
        nc.sync.dma_start(out=mask_sb, in_=mask[:, :])
        if has_bv:
            bv_bcast = bass.AP(tensor=bv.tensor if hasattr(bv, "tensor") else bv,
                               offset=0, ap=[[0, P], [1, HID]])
            nc.sync.dma_start(out=bv_sb, in_=bv_bcast)
        for c in range(NCH):
            nc.sync.dma_start(out=xT_c[c], in_=xT[c * P:(c + 1) * P, :])
        nc.sync.dma_start(out=wq_c[0], in_=wq[0])
        nc.sync.dma_start(out=wk_c[0], in_=wk[0])
        # remaining weight DMAs are staged into the early chunk slots,
        # ordered to mirror consumption
        late_dmas = [(wq_c[1], wq[1]), (wk_c[1], wk[1])]
        for c in range(NCH):
            late_dmas.append((wv_c[c], wv[c * P:(c + 1) * P, :]))
        for c in range(2, NCH):
            late_dmas.append((wq_c[c], wq[c]))
            late_dmas.append((wk_c[c], wk[c]))

        # ones columns for the softmax denominator live at col 64 of each
        # 65-wide head block; V copies below only overwrite cols 0..63
        nc.gpsimd.memset(v_sb, 1.0)

        # warmup matmuls on scratch data: keep the PE busy (clock ramp)
        # until xT + the first weight columns land
        wscr = persist.tile([P, 512], bf16, name="warm_scratch")
        nc.vector.memset(wscr, 0.5)
        for _ in range(16):
            wps = sc_ps.tile([P, S], fp32, name="score_psum")
            nc.tensor.matmul(
                wps[:, 0:512],
                lhsT=wscr[:, 0:P],
                rhs=wscr,
                start=True,
                stop=True,
            )

        qT_tiles = {}
        kT_tiles = {}  # kTp: [P, 2, S], head 2c+v at partitions 64v..64v+63
        qk_open = {}

        def qk_quarter(c, w_c, b_sb, dst_tiles, half, quarter):
            # one quarter (4 contraction chunks) of a Q/K projection half;
            # quarter 1 finishes the group and drains (+bias) to SBUF.
            # Quarters of one half are adjacent in the job list so the
            # qkv_ps rotation never sees two open groups.
            is_k = dst_tiles is kT_tiles
            if c not in dst_tiles:
                if is_k:
                    t = kT_pool.tile([P, 2, S], bf16, name="kTp")
                    # zero the pad rows (the other head's partitions) so the
                    # 128-row score contraction annihilates them
                    nc.gpsimd.memset(t[64:128, 0, :], 0.0)
                    nc.gpsimd.memset(t[0:64, 1, :], 0.0)
                    dst_tiles[c] = t
                else:
                    dst_tiles[c] = qT_pool.tile([P, S], bf16, name="qT")
            key = (id(dst_tiles), c, half)
            if quarter == 0:
                qk_open[key] = qkv_ps.tile([P, 512], fp32, name="qkv_psum")
            ps = qk_open[key]
            for kc in range(4 * quarter, 4 * quarter + 4):
                nc.tensor.matmul(
                    ps,
                    lhsT=w_c[c][:, kc, :],
                    rhs=xT_c[kc][:, half * 512:(half + 1) * 512],
                    start=(kc == 0),
                    stop=(kc == NCH - 1),
                )
            if quarter == 1:
                if is_k:
                    t = dst_tiles[c]
                    for v in range(2):
                        po = 64 * v
                        nc.vector.tensor_scalar_add(
                            out=t[po:po + 64, v, half * 512:(half + 1) * 512],
                            in0=ps[po:po + 64, :],
                            scalar1=b_sb[po:po + 64, c:c + 1],
                        )
                else:
                    nc.vector.tensor_scalar_add(
                        out=dst_tiles[c][:, half * 512:(half + 1) * 512],
                        in0=ps,
                        scalar1=b_sb[:, c:c + 1],
                    )
                del qk_open[key]

        def v_quarter(st, w):
            # v_sb[:, st, heads 4w..4w+3] = (X @ Wv)[st, 256-col quarter w]
            ps = qkv_ps.tile([P, 512], fp32, name="qkv_psum")[:, 0:256]
            for kc in range(NCH):
                nc.tensor.matmul(
                    ps,
                    lhsT=xT_c[kc][:, st * P:(st + 1) * P],
                    rhs=wv_c[kc][:, w * 256:(w + 1) * 256],
                    start=(kc == 0),
                    stop=(kc == NCH - 1),
                )
            dst = (
                v_sb[:, st, :]
                .rearrange("p (h x) -> p h x", x=HD + 1)[:, w * 4:(w + 1) * 4, 0:HD]
            )
            src = ps.rearrange("p (h x) -> p h x", x=HD)
            if has_bv:
                bvs = (
                    bv_sb[:, w * 256:(w + 1) * 256]
                    .rearrange("p (h x) -> p h x", x=HD)
                )
                nc.vector.tensor_add(out=dst, in0=src, in1=bvs)
            else:
                nc.vector.tensor_copy(out=dst, in_=src)

        def score_exp(c, sub, kt, pT_h):
            # one (head, kt) step: two 512-col padded-contraction matmuls
            # + exp. The sc pool's 2-buffer rotation within the sequential
            # per-head kt loop gives each matmul two exps of lookahead.
            ps = sc_ps.tile([P, S], fp32, name="score_psum")
            for half in range(2):
                nc.tensor.matmul(
                    ps[:, half * 512:(half + 1) * 512],
                    lhsT=kT_tiles[c][:, sub, kt * P:(kt + 1) * P],
                    rhs=qT_tiles[c][:, half * 512:(half + 1) * 512],
                    start=True,
                    stop=True,
                )
            nc.scalar.activation(
                out=pT_h[:, kt, :],
                in_=ps,
                func=EXP,
                bias=mask_sb[:, kt:kt + 1],
                scale=SCALE,
            )

        def ctx_quarter(h, pT_h, pair_out, col, qt_base):
            # two qt context groups + normalization for head h
            for qt in (qt_base, qt_base + 1):
                cps = cx_ps.tile([P, HD + 1], fp32, name="ctx_psum")
                for kc in range(NKT):
                    nc.tensor.matmul(
                        cps,
                        lhsT=pT_h[:, kc, qt * P:(qt + 1) * P],
                        rhs=v_sb[:, kc, h * (HD + 1):(h + 1) * (HD + 1)],
                        start=(kc == 0),
                        stop=(kc == NKT - 1),
                    )
                recip = misc.tile([P, 1], fp32, name="recip")
                nc.vector.reciprocal(recip, cps[:, HD:HD + 1])
                nc.vector.tensor_scalar_mul(
                    out=pair_out[:, qt, col * HD:(col + 1) * HD],
                    in0=cps[:, 0:HD],
                    scalar1=recip,
                )

        def ctx_chunk_jobs(cc):
            # 8 ctx quarter-jobs for head pair (2cc, 2cc+1) + final DMA
            pA, pB = pT_live.pop(cc)
            pair_out = out_pool.tile([P, NQT, 2 * HD], fp32, name="pair_out")
            jobs = []
            for qt_base in range(0, NQT, 2):
                jobs.append(("ctx", (2 * cc, pA, pair_out, 0, qt_base)))
                jobs.append(("ctx", (2 * cc + 1, pB, pair_out, 1, qt_base)))
            jobs.append(("dma", (cc, pair_out)))
            return jobs

        def pair_dma(cc, pair_out):
            # one 512KB DMA for the head pair's output columns
            dst = (
                out[:, 2 * cc * HD:(2 * cc + 2) * HD]
                .rearrange("(qt p) c -> p qt c", p=P)
            )
            nc.sync.dma_start(out=dst, in_=pair_out)

        def run_job(job):
            kind, args = job
            if kind == "qk":
                qk_quarter(*args)
            elif kind == "v":
                v_quarter(*args)
            elif kind == "ctx":
                ctx_quarter(*args)
            else:
                pair_dma(*args)

        # ---- pipeline ----
        # V width-quarter w feeds heads 4w..4w+3, first consumed by
        # ctx(2w) during chunk 2w+1 -> schedule strictly before that.
        v_sched = {
            0: [(st, 0) for st in range(NKT)],
            1: [(st, 1) for st in range(4)],
            2: [(st, 1) for st in range(4, NKT)],
            3: [(st, 2) for st in range(4)],
            4: [(st, 2) for st in range(4, NKT)],
            5: [(st, 3) for st in range(4)],
            6: [(st, 3) for st in range(4, NKT)],
        }
        pT_live = {}
        dma_stage = list(late_dmas)

        # Q(0) + K(0) half 0 ahead of the stream (head 2c's kts 0-3 only
        # need K h0; K h1 quarters are the first fillers of chunk 0)
        for half in range(2):
            for quarter in range(2):
                qk_quarter(0, wq_c, bq_sb, qT_tiles, half, quarter)
        for quarter in range(2):
            qk_quarter(0, wk_c, bk_sb, kT_tiles, 0, quarter)

        for c in range(NCH):
            pT_pair = (
                pT_pool.tile([P, NKT, S], bf16, name="pT"),
                pT_pool.tile([P, NKT, S], bf16, name="pT"),
            )
            pT_live[c] = pT_pair

            front = []  # spread over the first-half (head 2c) steps
            if c == 0:
                for quarter in range(2):
                    front.append(("qk", (0, wk_c, bk_sb, kT_tiles, 1, quarter)))
            if c + 1 < NCH:
                for w_c, b_sb, dst in ((wq_c, bq_sb, qT_tiles), (wk_c, bk_sb, kT_tiles)):
                    for half in range(2):
                        for quarter in range(2):
                            front.append(("qk", (c + 1, w_c, b_sb, dst, half, quarter)))
            v_c = [("v", vj) for vj in v_sched.get(c, [])]
            if c == 0:
                back = v_c  # wv DMAs stream in during the first half
            else:
                front.extend(v_c)
                back = ctx_chunk_jobs(c - 1)  # second-half (head 2c+1) steps

            # 16 per-chunk steps: head 2c kts 0-7, then head 2c+1 kts 0-7
            steps = [(0, kt) for kt in range(NKT)] + [(1, kt) for kt in range(NKT)]
            per_step = [[] for _ in range(16)]
            n = len(front)
            for i in range(8):
                per_step[i] = front[i * n // 8:(i + 1) * n // 8]
            nb = len(back)
            for i in range(8):
                per_step[8 + i] = back[i * nb // 8:(i + 1) * nb // 8]

            for idx, (sub, kt) in enumerate(steps):
                for _ in range(2):
                    if dma_stage:
                        dst, src_ap = dma_stage.pop(0)
                        nc.sync.dma_start(out=dst, in_=src_ap)
                score_exp(c, sub, kt, pT_pair[sub])
                for job in per_step[idx]:
                    run_job(job)
            qT_tiles.pop(c)
            kT_tiles.pop(c)

        # tail: last head pair
        for job in ctx_chunk_jobs(7):
            run_job(job)

    nc.finalize()
    return nc


def _prep_inputs(inputs):
    bf16 = ml_dtypes.bfloat16
    hs = np.asarray(inputs["hidden_states"], dtype=np.float32)
    am = np.asarray(inputs["attention_mask"], dtype=np.float32)
    Wq = np.asarray(inputs["Wq"], dtype=np.float32)
    Wk = np.asarray(inputs["Wk"], dtype=np.float32)
    Wv = np.asarray(inputs["Wv"], dtype=np.float32)
    bq = np.asarray(inputs["bq"], dtype=np.float32)
    bk = np.asarray(inputs["bk"], dtype=np.float32)
    bv = np.asarray(inputs["bv"], dtype=np.float32)

    has_bv = bool(np.any(bv))

    # [hid_in, hid_out] -> [c_out, p(hid_in%128), kc(hid_in//128), col]
    def col_shuffle(w):
        return np.ascontiguousarray(
            w.astype(bf16).reshape(NCH, P, NCH, P).transpose(2, 1, 0, 3)
        )

    wq_b = col_shuffle(Wq)
    wk_b = col_shuffle(Wk)
    wv_b = np.ascontiguousarray(Wv.astype(bf16))
    bq_c = np.ascontiguousarray(bq.reshape(NCH, P).T)
    bk_c = np.ascontiguousarray(bk.reshape(NCH, P).T)

    hs_b = hs.astype(bf16)
    in_maps = []
    for b in range(B):
        m = {
            "xT": np.ascontiguousarray(hs_b[b].T),
            "wq": wq_b,
            "wk": wk_b,
            "wv": wv_b,
            "bq": bq_c,
            "bk": bk_c,
            "mask": np.ascontiguousarray(am[b, 0, 0].reshape(NKT, P).T),
        }
        if has_bv:
            m["bv"] = bv
        in_maps.append(m)
    return in_maps, has_bv


def _run(inputs, trace=False, trace_cores=None):
    from concourse.bass_utils import run_bass_kernel_spmd

    in_maps, has_bv = _prep_inputs(inputs)
    nc = _build(has_bv)
    res = run_bass_kernel_spmd(
        nc, in_maps, core_ids=list(range(N_CORES)), trace=trace,
        trace_cores=trace_cores,
    )
    out = np.stack([np.asarray(r["out"], dtype=np.float32) for r in res.results])
    return out, res


def kernel(**inputs) -> np.ndarray:
    out, _ = _run(inputs, trace=False)
    return out
